# revision 17
# baseline (speedup 1.0000x reference)
"""Causal self-attention (B=2,T=2048,D=1024,H=16,HD=64) + RoPE on 8 TRN2 NeuronCores.

Sharding: core = b*4 + g  (b: batch, g: head-group of 4 heads).
Each core computes QKV projection for its 4 heads, causal attention, and a
partial out-projection (rank-256 contribution). Host sums the 4 partials per
batch (the "all-reduce after out_proj").

All matmul operands are bf16 (fp32 PSUM accumulation): full-rate PE rows,
fast weight loads, half the DMA bytes, and 2x/4x DVE elementwise rates.
The emission order software-pipelines QKV/out-proj matmul "filler" units
between attention (score -> exp -> PV) steps so the PE instruction stream
stays dense and HAM never re-throttles.
"""
from collections import deque

import numpy as np
import ml_dtypes

import concourse.bass as bass
import concourse.mybir as mybir
from concourse import bacc
from concourse.tile import TileContext
from concourse.bass_utils import run_bass_kernel_spmd

B, T, D, H = 2, 2048, 1024, 16
HD = D // H            # 64
G = 4                  # head groups (tensor-parallel factor)
HPG = H // G           # 4 heads per group
DG = HPG * HD          # 256 head-dims per group
KC = D // 128          # 8 contraction chunks for D
NT = T // 512          # 4 T-chunks of 512
TT = T // 128          # 16 T-tiles of 128
F32 = mybir.dt.float32
BF16 = mybir.dt.bfloat16
BF16_NP = ml_dtypes.bfloat16
SWAP16 = [(i + 16) % 32 for i in range(32)]  # e<->o halves within each 32-quadrant
N_WARM = 28
DEBUG_DUMPS = False

_CACHE = {}


def _build():
    nc = bacc.Bacc("TRN2", target_bir_lowering=False, debug=False, num_devices=8)

    xT_d = nc.dram_tensor("xT", [128, NT, KC, 512], BF16, kind="ExternalInput").ap()
    wqk_d = nc.dram_tensor("wqk", [128, KC, 2 * DG], BF16, kind="ExternalInput").ap()
    wv_d = nc.dram_tensor("wv", [128, KC, DG], BF16, kind="ExternalInput").ap()
    wout_d = nc.dram_tensor("wout", [128, 2, D], BF16, kind="ExternalInput").ap()
    cos_d = nc.dram_tensor("cos128", [128, T], BF16, kind="ExternalInput").ap()
    sin_d = nc.dram_tensor("sin128s", [128, T], BF16, kind="ExternalInput").ap()
    tri_d = nc.dram_tensor("tri", [128, 128], BF16, kind="ExternalInput").ap()
    out_d = nc.dram_tensor("out", [T, D], BF16, kind="ExternalOutput").ap()

    with TileContext(nc) as tc:
        with (
            tc.tile_pool(name="const", bufs=1) as cpool,
            tc.tile_pool(name="big", bufs=1) as big,
            tc.tile_pool(name="rope", bufs=3) as rope,
            tc.tile_pool(name="expp", bufs=4) as expp,
            tc.tile_pool(name="denp", bufs=2) as denp,
            tc.tile_pool(name="outp", bufs=3) as outp,
            tc.tile_pool(name="ps_mm", bufs=2, space="PSUM") as ps_mm,
            tc.tile_pool(name="ps_sc", bufs=2, space="PSUM") as ps_sc,
            tc.tile_pool(name="ps_pv", bufs=1, space="PSUM") as ps_pv,
        ):
            cos_sb = cpool.tile([128, T], BF16)
            sin_sb = cpool.tile([128, T], BF16)
            tri_sb = cpool.tile([128, 128], BF16)
            xT_sb = big.tile([128, NT, KC, 512], BF16)
            wqk_sb = big.tile([128, KC, 2 * DG], BF16)
            wv_sb = big.tile([128, KC, DG], BF16)
            wout_sb = big.tile([128, 2, D], BF16)
            # first q/k matmuls need wqk + xT chunk 0; RoPE needs cos/sin;
            # v tiles need wv.  Later chunks + wout stream behind.
            nc.sync.dma_start(out=wqk_sb[:], in_=wqk_d[:])
            nc.sync.dma_start(out=xT_sb[:, 0], in_=xT_d[:, 0])
            nc.sync.dma_start(out=wv_sb[:], in_=wv_d[:])
            nc.sync.dma_start(out=cos_sb[:], in_=cos_d[:])
            nc.sync.dma_start(out=sin_sb[:], in_=sin_d[:])
            nc.sync.dma_start(out=tri_sb[:], in_=tri_d[:])
            for n in range(1, NT):
                nc.sync.dma_start(out=xT_sb[:, n], in_=xT_d[:, n])
            nc.sync.dma_start(out=wout_sb[:], in_=wout_d[:])

            # PE warm-up: dummy matmuls fill the DMA lead-in so HAM unthrottles
            # before the first real matmul
            warm_sb = cpool.tile([128, 256], BF16)
            nc.vector.memset(warm_sb[:], 0.0)
            for w in range(N_WARM):
                wp = ps_sc.tile([128, 256], F32, tag="sc")
                nc.tensor.matmul(
                    wp[:], lhsT=warm_sb[:, 0:128], rhs=warm_sb[:],
                    start=True, stop=True,
                )

            # qkT_sb m-index: 0,1 = q head-pairs (0,1),(2,3); 2,3 = k pairs
            qkT_sb = big.tile([128, 4, T], BF16)
            v_sb = big.tile([128, TT, HPG, HD + 1], BF16)
            nc.vector.memset(v_sb[:, :, :, HD], 1.0)
            outT_sb = big.tile([128, 2, T], BF16)

            def qk_tile(n, m):
                """q/k projection tile m (columns 512n:512n+512) + RoPE.
                Yields once mid-way (filler segmentation)."""
                ns = slice(n * 512, (n + 1) * 512)
                ps = ps_mm.tile([128, 512], F32, tag="mm")
                for k in range(KC):
                    nc.tensor.matmul(
                        ps[:],
                        lhsT=wqk_sb[:, k, m * 128:(m + 1) * 128],
                        rhs=xT_sb[:, n, k, :],
                        start=(k == 0),
                        stop=(k == KC - 1),
                    )
                    if k == 3:
                        yield
                # RoPE: rot = ps*cos + swap16(ps)*sin_signed
                qk_bf = rope.tile([128, 512], BF16, tag="qkbf")
                nc.scalar.copy(out=qk_bf[:], in_=ps[:])
                swp = rope.tile([128, 512], BF16, tag="swp")
                nc.vector.stream_shuffle(swp[:], qk_bf[:], SWAP16)
                m1 = rope.tile([128, 512], BF16, tag="m1")
                nc.vector.tensor_mul(m1[:], qk_bf[:], cos_sb[:, ns])
                m2 = rope.tile([128, 512], BF16, tag="m2")
                nc.gpsimd.tensor_mul(m2[:], swp[:], sin_sb[:, ns])
                nc.vector.tensor_add(qkT_sb[:, m, ns], m1[:], m2[:])

            def v_tile(j):
                """v projection for T-tile j (natural layout)."""
                n = j // 4
                ps = ps_mm.tile([128, 256], F32, tag="mm")
                for k in range(KC):
                    nc.tensor.matmul(
                        ps[:],
                        lhsT=xT_sb[:, n, k, (j % 4) * 128:(j % 4 + 1) * 128],
                        rhs=wv_sb[:, k, :],
                        start=(k == 0),
                        stop=(k == KC - 1),
                    )
                vv = ps[:].rearrange("p (h d) -> p h d", h=HPG)
                if j % 2 == 0:
                    nc.scalar.copy(out=v_sb[:, j, :, 0:HD], in_=vv)
                else:
                    nc.vector.tensor_copy(v_sb[:, j, :, 0:HD], vv)
                yield

            def proj_tile(t, nh):
                """out-projection partial for T-tile t, D-half nh."""
                ps = ps_mm.tile([128, 512], F32, tag="mm")
                for c in range(2):
                    nc.tensor.matmul(
                        ps[:],
                        lhsT=outT_sb[:, c, t * 128:(t + 1) * 128],
                        rhs=wout_sb[:, c, nh * 512:(nh + 1) * 512],
                        start=(c == 0),
                        stop=(c == 1),
                    )
                ot = outp.tile([128, 512], BF16, tag="ot")
                if t < 8 and (t + nh) % 2 == 0:
                    nc.scalar.copy(out=ot[:], in_=ps[:])
                else:
                    nc.vector.tensor_copy(ot[:], ps[:])
                nc.sync.dma_start(
                    out=out_d[t * 128:(t + 1) * 128, nh * 512:(nh + 1) * 512],
                    in_=ot[:],
                )
                yield

            fillers = deque()

            def drain(k):
                while k > 0 and fillers:
                    gen = fillers.popleft()
                    try:
                        next(gen)
                        fillers.appendleft(gen)
                    except StopIteration:
                        pass
                    k -= 1

            def drain_all():
                while fillers:
                    drain(1)

            def attn_group(g):
                """Attention for query block g (queries 512g:512g+512), with
                filler units interleaved between steps.  PV emission lags two
                steps behind scores so the PE never waits on exp, and filler
                draining is front-loaded so cross-group RoPE chains finish
                early and group boundaries have PE food during the den path."""
                steps = [(hp, j) for hp in range(2) for j in range(4 * g + 4)]
                quota, acc = len(fillers) / len(steps), 0.0
                pend = deque()
                pvt = {}
                for hp, j in steps:
                    if j == 0:
                        pvt[0] = ps_pv.tile([65, 512], F32, tag="pv0", name="pv0")
                        pvt[1] = ps_pv.tile([65, 512], F32, tag="pv1", name="pv1")
                    d = j - 4 * g
                    nstart = 128 * d if d > 0 else 0
                    ncols = 512 - nstart
                    jmax = 4 * g + 3
                    sc = ps_sc.tile([128, 1024], F32, tag="sc")
                    ex = expp.tile([128, 1024], BF16, tag="ex")
                    # two heads' score matmuls (row groups 0-1 / 2-3), one
                    # wide exp over both
                    for half in range(2):
                        pb = 64 * half
                        nc.tensor.matmul(
                            sc[:, half * 512:half * 512 + ncols],
                            lhsT=qkT_sb[pb:pb + 64, 2 + hp, j * 128:(j + 1) * 128],
                            rhs=qkT_sb[pb:pb + 64, hp, g * 512 + nstart:(g + 1) * 512],
                            start=True,
                            stop=True,
                        )
                    if ncols == 512:
                        nc.scalar.activation(
                            ex[:], sc[:],
                            mybir.ActivationFunctionType.Exp, scale=0.125,
                        )
                    else:
                        exv = ex[:].rearrange("p (u c) -> p u c", u=2)[:, :, 0:ncols]
                        scv = sc[:].rearrange("p (u c) -> p u c", u=2)[:, :, 0:ncols]
                        nc.scalar.activation(
                            exv, scv, mybir.ActivationFunctionType.Exp, scale=0.125,
                        )
                    if d >= 0:
                        nc.vector.tensor_mul(ex[:, 0:128], ex[:, 0:128], tri_sb[:])
                        nc.gpsimd.tensor_mul(ex[:, 512:640], ex[:, 512:640], tri_sb[:])

                    acc += quota
                    while acc >= 1.0:
                        drain(1)
                        acc -= 1.0

                    def emit_pv(hp, j, ex, nstart, ncols):
                        for half in range(2):
                            nc.tensor.matmul(
                                pvt[half][:, nstart:512],
                                lhsT=v_sb[:, j, 2 * hp + half, :],
                                rhs=ex[:, half * 512:half * 512 + ncols],
                                start=(j == 0),
                                stop=(j == jmax),
                            )

                    if len(pend) == 2:
                        emit_pv(*pend.popleft())
                    pend.append((hp, j, ex, nstart, ncols))
                    if j == jmax:
                        while pend:
                            drain(1)
                            emit_pv(*pend.popleft())
                        # normalize: denominators sit in pv row 64 (ones col)
                        for half in range(2):
                            pb = 64 * half
                            pv = pvt[half]
                            den = denp.tile([1, 512], F32, tag="den")
                            nc.scalar.copy(out=den[:], in_=pv[64:65, :])
                            rec = denp.tile([1, 512], F32, tag="rec")
                            nc.vector.reciprocal_approx_fast(rec[:], den[:])
                            recb = denp.tile([64, 512], F32, tag="recb")
                            nc.gpsimd.partition_broadcast(recb[:], rec[0:1, :], channels=64)
                            nc.vector.tensor_mul(
                                outT_sb[pb:pb + 64, hp, g * 512:(g + 1) * 512],
                                pv[0:64, :],
                                recb[:],
                            )
                        drain(2)
                drain_all()

            # preamble: chunk-0 q/k pairs needed by attn0-hp0 first, then v 0-3
            # and the hp1 pairs (their PE work covers m0/m2's RoPE-chain latency)
            for gen in [qk_tile(0, 0), qk_tile(0, 2), v_tile(0), v_tile(1),
                        v_tile(2), v_tile(3), qk_tile(0, 1), qk_tile(0, 3)]:
                for _ in gen:
                    pass

            # attn0 fillers: chunk1 qk
            fillers.extend(qk_tile(1, m) for m in range(4))
            attn_group(0)

            # attn1 fillers: v 4-7 first (needed by attn1 pv), chunk2, proj0
            fillers.extend(v_tile(j) for j in range(4, 8))
            fillers.extend(qk_tile(2, m) for m in range(4))
            fillers.extend(proj_tile(t, nh) for t in range(0, 4) for nh in range(2))
            attn_group(1)

            # attn2 fillers: v 8-11, chunk3, proj1
            fillers.extend(v_tile(j) for j in range(8, 12))
            fillers.extend(qk_tile(3, m) for m in range(4))
            fillers.extend(proj_tile(t, nh) for t in range(4, 8) for nh in range(2))
            attn_group(2)

            # attn3 fillers: v 12-15, proj2, proj3
            fillers.extend(v_tile(j) for j in range(12, 16))
            fillers.extend(proj_tile(t, nh) for t in range(8, 12) for nh in range(2))
            attn_group(3)

            fillers.extend(proj_tile(t, nh) for t in range(12, 16) for nh in range(2))
            drain_all()

            if DEBUG_DUMPS:
                dq = nc.dram_tensor("dbg_qk", [128, 4, T], BF16, kind="ExternalOutput").ap()
                dv = nc.dram_tensor("dbg_v", [128, TT, HPG, HD + 1], BF16, kind="ExternalOutput").ap()
                do = nc.dram_tensor("dbg_outT", [128, 2, T], BF16, kind="ExternalOutput").ap()
                nc.sync.dma_start(out=dq[:], in_=qkT_sb[:])
                nc.sync.dma_start(out=dv[:], in_=v_sb[:])
                nc.sync.dma_start(out=do[:], in_=outT_sb[:])

    nc.compile()
    return nc


def _qk_perm():
    """hd permutation for q/k columns: RoPE pair j -> (e,o) rows 16-interleaved
    so the swap stays within 32-partition quadrants (stream_shuffle-able)."""
    perm = np.empty(HD, dtype=np.int64)
    for p in range(HD):
        q32, i = divmod(p, 32)
        j = 16 * q32 + (i % 16)
        perm[p] = 2 * j + (1 if i >= 16 else 0)
    return perm


def _prepare_shards(x, w_qkv, w_out, freqs_cos, freqs_sin):
    perm = _qk_perm()
    cosT = np.ascontiguousarray(freqs_cos.T)  # [32, T]
    sinT = np.ascontiguousarray(freqs_sin.T)
    # row p of a 64-row head block: pair j = 16*(p//32 % 2) + p%16, sign -/+ for e/o
    cos128 = np.empty((128, T), dtype=np.float32)
    sin128s = np.empty((128, T), dtype=np.float32)
    for p in range(128):
        ph = p % 64
        q32, i = divmod(ph, 32)
        j = 16 * q32 + (i % 16)
        cos128[p] = cosT[j]
        sin128s[p] = sinT[j] * (-1.0 if i < 16 else 1.0)
    kk, qq = np.meshgrid(np.arange(128), np.arange(128), indexing="ij")
    tri = (kk <= qq).astype(np.float32)

    w3 = w_qkv.reshape(D, 3, H, HD)
    in_maps = []
    for core in range(8):
        b, g = divmod(core, G)
        heads = np.arange(g * HPG, (g + 1) * HPG)
        wq = w3[:, 0, heads][:, :, perm].reshape(D, DG)
        wk = w3[:, 1, heads][:, :, perm].reshape(D, DG)
        wqk = np.ascontiguousarray(np.concatenate([wq, wk], axis=1))
        wv = np.ascontiguousarray(w3[:, 2, heads].reshape(D, DG))
        wo = np.ascontiguousarray(w_out.reshape(H, HD, D)[heads].reshape(DG, D))
        def sb_layout(a, kc=KC):
            # [128*kc, F] -> [128, kc, F] with partition-major contiguity
            return np.ascontiguousarray(a.reshape(kc, 128, -1).transpose(1, 0, 2))
        # xT chunks: [128, NT, KC, 512] so each chunk is one contiguous DMA
        xc = x[b].reshape(NT, 512, KC, 128).transpose(3, 0, 2, 1)
        in_maps.append({
            "xT": np.ascontiguousarray(xc).astype(BF16_NP),
            "wqk": sb_layout(wqk).astype(BF16_NP),
            "wv": sb_layout(wv).astype(BF16_NP),
            "wout": sb_layout(wo, kc=2).astype(BF16_NP),
            "cos128": cos128.astype(BF16_NP),
            "sin128s": sin128s.astype(BF16_NP),
            "tri": tri.astype(BF16_NP),
        })
    return in_maps


def _run(in_maps, **kw):
    if "nc" not in _CACHE:
        _CACHE["nc"] = _build()
    return run_bass_kernel_spmd(_CACHE["nc"], in_maps, core_ids=list(range(8)), **kw)


def kernel(x, w_qkv, w_out, freqs_cos, freqs_sin):
    x = np.asarray(x, dtype=np.float32)
    w_qkv = np.asarray(w_qkv, dtype=np.float32)
    w_out = np.asarray(w_out, dtype=np.float32)
    freqs_cos = np.asarray(freqs_cos, dtype=np.float32)
    freqs_sin = np.asarray(freqs_sin, dtype=np.float32)

    in_maps = _prepare_shards(x, w_qkv, w_out, freqs_cos, freqs_sin)
    res = _run(in_maps)
    out = np.zeros((B, T, D), dtype=np.float64)
    for core in range(8):
        out[core // G] += np.asarray(res.results[core]["out"]).astype(np.float64)
    return out.astype(np.float32)


# revision 19
# speedup vs baseline: 1.2745x; 1.2745x over previous
"""Causal self-attention (B=2,T=2048,D=1024,H=16,HD=64) + RoPE on 8 TRN2 NeuronCores.

Sharding: core = b*4 + g  (b: batch, g: head-group of 4 heads).
Each core computes QKV projection for its 4 heads, causal attention, and a
partial out-projection (rank-256 contribution). Host sums the 4 partials per
batch (the "all-reduce after out_proj").

All matmul operands are bf16 (fp32 PSUM accumulation): full-rate PE rows,
fast weight loads, half the DMA bytes, and 2x/4x DVE elementwise rates.
The emission order software-pipelines QKV/out-proj matmul "filler" units
between attention (score -> exp -> PV) steps so the PE instruction stream
stays dense and HAM never re-throttles.
"""
from collections import deque

import numpy as np
import ml_dtypes

import concourse.bass as bass
import concourse.mybir as mybir
from concourse import bacc
from concourse.tile import TileContext
from concourse.bass_utils import run_bass_kernel_spmd

B, T, D, H = 2, 2048, 1024, 16
HD = D // H            # 64
G = 4                  # head groups (tensor-parallel factor)
HPG = H // G           # 4 heads per group
DG = HPG * HD          # 256 head-dims per group
KC = D // 128          # 8 contraction chunks for D
NT = T // 512          # 4 T-chunks of 512
TT = T // 128          # 16 T-tiles of 128
F32 = mybir.dt.float32
BF16 = mybir.dt.bfloat16
BF16_NP = ml_dtypes.bfloat16
SWAP16 = [(i + 16) % 32 for i in range(32)]  # e<->o halves within each 32-quadrant
N_WARM = 28
DEBUG_DUMPS = False

_CACHE = {}


def _build():
    nc = bacc.Bacc("TRN2", target_bir_lowering=False, debug=False, num_devices=8)

    xT_d = nc.dram_tensor("xT", [128, NT, KC, 512], BF16, kind="ExternalInput").ap()
    wqk_d = nc.dram_tensor("wqk", [128, KC, 2 * DG], BF16, kind="ExternalInput").ap()
    wv_d = nc.dram_tensor("wv", [128, KC, DG], BF16, kind="ExternalInput").ap()
    wout_d = nc.dram_tensor("wout", [128, 2, D], BF16, kind="ExternalInput").ap()
    cos_d = nc.dram_tensor("cos128", [128, T], BF16, kind="ExternalInput").ap()
    sin_d = nc.dram_tensor("sin128s", [128, T], BF16, kind="ExternalInput").ap()
    tri_d = nc.dram_tensor("tri", [128, 128], BF16, kind="ExternalInput").ap()
    out_d = nc.dram_tensor("out", [T, D], BF16, kind="ExternalOutput").ap()

    with TileContext(nc) as tc:
        with (
            tc.tile_pool(name="const", bufs=1) as cpool,
            tc.tile_pool(name="big", bufs=1) as big,
            tc.tile_pool(name="rope", bufs=3) as rope,
            tc.tile_pool(name="expp", bufs=4) as expp,
            tc.tile_pool(name="denp", bufs=2) as denp,
            tc.tile_pool(name="outp", bufs=3) as outp,
            tc.tile_pool(name="ps_mm", bufs=2, space="PSUM") as ps_mm,
            tc.tile_pool(name="ps_sc", bufs=2, space="PSUM") as ps_sc,
            tc.tile_pool(name="ps_pv", bufs=1, space="PSUM") as ps_pv,
        ):
            cos_sb = cpool.tile([128, T], BF16)
            sin_sb = cpool.tile([128, T], BF16)
            tri_sb = cpool.tile([128, 128], BF16)
            xT_sb = big.tile([128, NT, KC, 512], BF16)
            wqk_sb = big.tile([128, KC, 2 * DG], BF16)
            wv_sb = big.tile([128, KC, DG], BF16)
            wout_sb = big.tile([128, 2, D], BF16)
            # first q/k matmuls need wqk + xT chunk 0; RoPE needs cos/sin;
            # v tiles need wv.  Later chunks + wout stream behind.
            nc.sync.dma_start(out=wqk_sb[:], in_=wqk_d[:])
            nc.sync.dma_start(out=xT_sb[:, 0], in_=xT_d[:, 0])
            nc.sync.dma_start(out=wv_sb[:], in_=wv_d[:])
            nc.sync.dma_start(out=cos_sb[:], in_=cos_d[:])
            nc.sync.dma_start(out=sin_sb[:], in_=sin_d[:])
            nc.sync.dma_start(out=tri_sb[:], in_=tri_d[:])
            for n in range(1, NT):
                nc.sync.dma_start(out=xT_sb[:, n], in_=xT_d[:, n])
            nc.sync.dma_start(out=wout_sb[:], in_=wout_d[:])

            # PE warm-up: dummy matmuls fill the DMA lead-in so HAM unthrottles
            # before the first real matmul
            warm_sb = cpool.tile([128, 256], BF16)
            nc.vector.memset(warm_sb[:], 0.0)
            for w in range(N_WARM):
                wp = ps_sc.tile([128, 256], F32, tag="sc")
                nc.tensor.matmul(
                    wp[:], lhsT=warm_sb[:, 0:128], rhs=warm_sb[:],
                    start=True, stop=True,
                )

            # qkT_sb m-index: 0,1 = q head-pairs (0,1),(2,3); 2,3 = k pairs
            qkT_sb = big.tile([128, 4, T], BF16)
            v_sb = big.tile([128, TT, HPG, HD + 1], BF16)
            nc.vector.memset(v_sb[:, :, :, HD], 1.0)
            outT_sb = big.tile([128, 2, T], BF16)

            def qk_tile(n, m):
                """q/k projection tile m (columns 512n:512n+512) + RoPE.
                Yields once mid-way (filler segmentation)."""
                ns = slice(n * 512, (n + 1) * 512)
                ps = ps_mm.tile([128, 512], F32, tag="mm")
                for k in range(KC):
                    nc.tensor.matmul(
                        ps[:],
                        lhsT=wqk_sb[:, k, m * 128:(m + 1) * 128],
                        rhs=xT_sb[:, n, k, :],
                        start=(k == 0),
                        stop=(k == KC - 1),
                    )
                    if k == 3:
                        yield
                # RoPE: rot = ps*cos + swap16(ps)*sin_signed
                qk_bf = rope.tile([128, 512], BF16, tag="qkbf")
                nc.scalar.copy(out=qk_bf[:], in_=ps[:])
                swp = rope.tile([128, 512], BF16, tag="swp")
                nc.vector.stream_shuffle(swp[:], qk_bf[:], SWAP16)
                m1 = rope.tile([128, 512], BF16, tag="m1")
                nc.vector.tensor_mul(m1[:], qk_bf[:], cos_sb[:, ns])
                m2 = rope.tile([128, 512], BF16, tag="m2")
                nc.gpsimd.tensor_mul(m2[:], swp[:], sin_sb[:, ns])
                nc.vector.tensor_add(qkT_sb[:, m, ns], m1[:], m2[:])

            def v_tile(j):
                """v projection for T-tile j (natural layout)."""
                n = j // 4
                ps = ps_mm.tile([128, 256], F32, tag="mm")
                for k in range(KC):
                    nc.tensor.matmul(
                        ps[:],
                        lhsT=xT_sb[:, n, k, (j % 4) * 128:(j % 4 + 1) * 128],
                        rhs=wv_sb[:, k, :],
                        start=(k == 0),
                        stop=(k == KC - 1),
                    )
                vv = ps[:].rearrange("p (h d) -> p h d", h=HPG)
                if j % 2 == 0:
                    nc.scalar.copy(out=v_sb[:, j, :, 0:HD], in_=vv)
                else:
                    nc.vector.tensor_copy(v_sb[:, j, :, 0:HD], vv)
                yield

            def proj_tile(t, nh):
                """out-projection partial for T-tile t, D-half nh."""
                ps = ps_mm.tile([128, 512], F32, tag="mm")
                for c in range(2):
                    nc.tensor.matmul(
                        ps[:],
                        lhsT=outT_sb[:, c, t * 128:(t + 1) * 128],
                        rhs=wout_sb[:, c, nh * 512:(nh + 1) * 512],
                        start=(c == 0),
                        stop=(c == 1),
                    )
                ot = outp.tile([128, 512], BF16, tag="ot")
                if t < 8 and (t + nh) % 2 == 0:
                    nc.scalar.copy(out=ot[:], in_=ps[:])
                else:
                    nc.vector.tensor_copy(ot[:], ps[:])
                nc.sync.dma_start(
                    out=out_d[t * 128:(t + 1) * 128, nh * 512:(nh + 1) * 512],
                    in_=ot[:],
                )
                yield

            fillers = deque()

            def drain(k):
                while k > 0 and fillers:
                    gen = fillers.popleft()
                    try:
                        next(gen)
                        fillers.appendleft(gen)
                    except StopIteration:
                        pass
                    k -= 1

            def drain_all():
                while fillers:
                    drain(1)

            def attn_group(g):
                """Attention for query block g (queries 512g:512g+512), with
                filler units interleaved between steps.  PV emission lags two
                steps behind scores so the PE never waits on exp, and filler
                draining is front-loaded so cross-group RoPE chains finish
                early and group boundaries have PE food during the den path."""
                steps = [(hp, j) for hp in range(2) for j in range(4 * g + 4)]
                quota, acc = len(fillers) / len(steps), 0.0
                pend = deque()
                pvt = {}
                for hp, j in steps:
                    if j == 0:
                        pvt[0] = ps_pv.tile([65, 512], F32, tag="pv0", name="pv0")
                        pvt[1] = ps_pv.tile([65, 512], F32, tag="pv1", name="pv1")
                    d = j - 4 * g
                    nstart = 128 * d if d > 0 else 0
                    ncols = 512 - nstart
                    jmax = 4 * g + 3
                    sc = ps_sc.tile([128, 1024], F32, tag="sc")
                    ex = expp.tile([128, 1024], BF16, tag="ex")
                    # two heads' score matmuls (row groups 0-1 / 2-3), one
                    # wide exp over both
                    for half in range(2):
                        pb = 64 * half
                        nc.tensor.matmul(
                            sc[:, half * 512:half * 512 + ncols],
                            lhsT=qkT_sb[pb:pb + 64, 2 + hp, j * 128:(j + 1) * 128],
                            rhs=qkT_sb[pb:pb + 64, hp, g * 512 + nstart:(g + 1) * 512],
                            start=True,
                            stop=True,
                        )
                    if ncols == 512:
                        nc.scalar.activation(
                            ex[:], sc[:],
                            mybir.ActivationFunctionType.Exp, scale=0.125,
                        )
                    else:
                        exv = ex[:].rearrange("p (u c) -> p u c", u=2)[:, :, 0:ncols]
                        scv = sc[:].rearrange("p (u c) -> p u c", u=2)[:, :, 0:ncols]
                        nc.scalar.activation(
                            exv, scv, mybir.ActivationFunctionType.Exp, scale=0.125,
                        )
                    if d >= 0:
                        nc.vector.tensor_mul(ex[:, 0:128], ex[:, 0:128], tri_sb[:])
                        nc.vector.tensor_mul(ex[:, 512:640], ex[:, 512:640], tri_sb[:])

                    acc += quota
                    while acc >= 1.0:
                        drain(1)
                        acc -= 1.0

                    def emit_pv(hp, j, ex, nstart, ncols):
                        for half in range(2):
                            nc.tensor.matmul(
                                pvt[half][:, nstart:512],
                                lhsT=v_sb[:, j, 2 * hp + half, :],
                                rhs=ex[:, half * 512:half * 512 + ncols],
                                start=(j == 0),
                                stop=(j == jmax),
                            )

                    if len(pend) == 2:
                        emit_pv(*pend.popleft())
                    pend.append((hp, j, ex, nstart, ncols))
                    if j == jmax:
                        while pend:
                            drain(1)
                            emit_pv(*pend.popleft())
                        # normalize: denominators sit in pv row 64 (ones col)
                        for half in range(2):
                            pb = 64 * half
                            pv = pvt[half]
                            den = denp.tile([1, 512], F32, tag="den")
                            nc.vector.tensor_copy(den[:], pv[64:65, :])
                            rec = denp.tile([1, 512], F32, tag="rec")
                            nc.vector.reciprocal_approx_fast(rec[:], den[:])
                            recb = denp.tile([64, 512], F32, tag="recb")
                            nc.gpsimd.partition_broadcast(recb[:], rec[0:1, :], channels=64)
                            nc.vector.tensor_mul(
                                outT_sb[pb:pb + 64, hp, g * 512:(g + 1) * 512],
                                pv[0:64, :],
                                recb[:],
                            )
                        drain(2)
                drain_all()

            # preamble: chunk-0 q/k pairs needed by attn0-hp0 first, then v 0-3
            # and the hp1 pairs (their PE work covers m0/m2's RoPE-chain latency)
            for gen in [qk_tile(0, 0), qk_tile(0, 2), v_tile(0), v_tile(1),
                        v_tile(2), v_tile(3), qk_tile(0, 1), qk_tile(0, 3)]:
                for _ in gen:
                    pass

            # attn0 fillers: chunk1 qk
            fillers.extend(qk_tile(1, m) for m in range(4))
            attn_group(0)

            # attn1 fillers: v 4-7 first (needed by attn1 pv), chunk2, proj0
            fillers.extend(v_tile(j) for j in range(4, 8))
            fillers.extend(qk_tile(2, m) for m in range(4))
            fillers.extend(proj_tile(t, nh) for t in range(0, 4) for nh in range(2))
            attn_group(1)

            # attn2 fillers: v 8-11, chunk3, proj1
            fillers.extend(v_tile(j) for j in range(8, 12))
            fillers.extend(qk_tile(3, m) for m in range(4))
            fillers.extend(proj_tile(t, nh) for t in range(4, 8) for nh in range(2))
            attn_group(2)

            # attn3 fillers: v 12-15, proj2, proj3
            fillers.extend(v_tile(j) for j in range(12, 16))
            fillers.extend(proj_tile(t, nh) for t in range(8, 12) for nh in range(2))
            attn_group(3)

            fillers.extend(proj_tile(t, nh) for t in range(12, 16) for nh in range(2))
            drain_all()

            if DEBUG_DUMPS:
                dq = nc.dram_tensor("dbg_qk", [128, 4, T], BF16, kind="ExternalOutput").ap()
                dv = nc.dram_tensor("dbg_v", [128, TT, HPG, HD + 1], BF16, kind="ExternalOutput").ap()
                do = nc.dram_tensor("dbg_outT", [128, 2, T], BF16, kind="ExternalOutput").ap()
                nc.sync.dma_start(out=dq[:], in_=qkT_sb[:])
                nc.sync.dma_start(out=dv[:], in_=v_sb[:])
                nc.sync.dma_start(out=do[:], in_=outT_sb[:])

    nc.compile()
    return nc


def _qk_perm():
    """hd permutation for q/k columns: RoPE pair j -> (e,o) rows 16-interleaved
    so the swap stays within 32-partition quadrants (stream_shuffle-able)."""
    perm = np.empty(HD, dtype=np.int64)
    for p in range(HD):
        q32, i = divmod(p, 32)
        j = 16 * q32 + (i % 16)
        perm[p] = 2 * j + (1 if i >= 16 else 0)
    return perm


def _prepare_shards(x, w_qkv, w_out, freqs_cos, freqs_sin):
    perm = _qk_perm()
    cosT = np.ascontiguousarray(freqs_cos.T)  # [32, T]
    sinT = np.ascontiguousarray(freqs_sin.T)
    # row p of a 64-row head block: pair j = 16*(p//32 % 2) + p%16, sign -/+ for e/o
    cos128 = np.empty((128, T), dtype=np.float32)
    sin128s = np.empty((128, T), dtype=np.float32)
    for p in range(128):
        ph = p % 64
        q32, i = divmod(ph, 32)
        j = 16 * q32 + (i % 16)
        cos128[p] = cosT[j]
        sin128s[p] = sinT[j] * (-1.0 if i < 16 else 1.0)
    kk, qq = np.meshgrid(np.arange(128), np.arange(128), indexing="ij")
    tri = (kk <= qq).astype(np.float32)

    w3 = w_qkv.reshape(D, 3, H, HD)
    in_maps = []
    for core in range(8):
        b, g = divmod(core, G)
        heads = np.arange(g * HPG, (g + 1) * HPG)
        wq = w3[:, 0, heads][:, :, perm].reshape(D, DG)
        wk = w3[:, 1, heads][:, :, perm].reshape(D, DG)
        wqk = np.ascontiguousarray(np.concatenate([wq, wk], axis=1))
        wv = np.ascontiguousarray(w3[:, 2, heads].reshape(D, DG))
        wo = np.ascontiguousarray(w_out.reshape(H, HD, D)[heads].reshape(DG, D))
        def sb_layout(a, kc=KC):
            # [128*kc, F] -> [128, kc, F] with partition-major contiguity
            return np.ascontiguousarray(a.reshape(kc, 128, -1).transpose(1, 0, 2))
        # xT chunks: [128, NT, KC, 512] so each chunk is one contiguous DMA
        xc = x[b].reshape(NT, 512, KC, 128).transpose(3, 0, 2, 1)
        in_maps.append({
            "xT": np.ascontiguousarray(xc).astype(BF16_NP),
            "wqk": sb_layout(wqk).astype(BF16_NP),
            "wv": sb_layout(wv).astype(BF16_NP),
            "wout": sb_layout(wo, kc=2).astype(BF16_NP),
            "cos128": cos128.astype(BF16_NP),
            "sin128s": sin128s.astype(BF16_NP),
            "tri": tri.astype(BF16_NP),
        })
    return in_maps


def _run(in_maps, **kw):
    if "nc" not in _CACHE:
        _CACHE["nc"] = _build()
    return run_bass_kernel_spmd(_CACHE["nc"], in_maps, core_ids=list(range(8)), **kw)


def kernel(x, w_qkv, w_out, freqs_cos, freqs_sin):
    x = np.asarray(x, dtype=np.float32)
    w_qkv = np.asarray(w_qkv, dtype=np.float32)
    w_out = np.asarray(w_out, dtype=np.float32)
    freqs_cos = np.asarray(freqs_cos, dtype=np.float32)
    freqs_sin = np.asarray(freqs_sin, dtype=np.float32)

    in_maps = _prepare_shards(x, w_qkv, w_out, freqs_cos, freqs_sin)
    res = _run(in_maps)
    out = np.zeros((B, T, D), dtype=np.float64)
    for core in range(8):
        out[core // G] += np.asarray(res.results[core]["out"]).astype(np.float64)
    return out.astype(np.float32)


# revision 22
# speedup vs baseline: 1.6941x; 1.3293x over previous
"""Causal self-attention (B=2,T=2048,D=1024,H=16,HD=64) + RoPE on 8 TRN2 NeuronCores.

Sharding: core = b*4 + g  (b: batch, g: head-group of 4 heads).
Each core computes QKV projection for its 4 heads, causal attention, and a
partial out-projection (rank-256 contribution). Host sums the 4 partials per
batch (the "all-reduce after out_proj").

All matmul operands are bf16 (fp32 PSUM accumulation): full-rate PE rows,
fast weight loads, half the DMA bytes, and 2x/4x DVE elementwise rates.
The emission order software-pipelines QKV/out-proj matmul "filler" units
between attention (score -> exp -> PV) steps so the PE instruction stream
stays dense and HAM never re-throttles.
"""
from collections import deque

import numpy as np
import ml_dtypes

import concourse.bass as bass
import concourse.mybir as mybir
from concourse import bacc
from concourse.tile import TileContext
from concourse.bass_utils import run_bass_kernel_spmd

B, T, D, H = 2, 2048, 1024, 16
HD = D // H            # 64
G = 4                  # head groups (tensor-parallel factor)
HPG = H // G           # 4 heads per group
DG = HPG * HD          # 256 head-dims per group
KC = D // 128          # 8 contraction chunks for D
NT = T // 512          # 4 T-chunks of 512
TT = T // 128          # 16 T-tiles of 128
F32 = mybir.dt.float32
BF16 = mybir.dt.bfloat16
BF16_NP = ml_dtypes.bfloat16
SWAP16 = [(i + 16) % 32 for i in range(32)]  # e<->o halves within each 32-quadrant
N_WARM = 28
DEBUG_DUMPS = False

_CACHE = {}


def _build():
    nc = bacc.Bacc("TRN2", target_bir_lowering=False, debug=False, num_devices=8)

    xT_d = nc.dram_tensor("xT", [128, NT, KC, 512], BF16, kind="ExternalInput").ap()
    wqk_d = nc.dram_tensor("wqk", [128, KC, 2 * DG], BF16, kind="ExternalInput").ap()
    wv_d = nc.dram_tensor("wv", [128, KC, DG], BF16, kind="ExternalInput").ap()
    wout_d = nc.dram_tensor("wout", [128, 2, D], BF16, kind="ExternalInput").ap()
    cos_d = nc.dram_tensor("cos128", [128, T], BF16, kind="ExternalInput").ap()
    sin_d = nc.dram_tensor("sin128s", [128, T], BF16, kind="ExternalInput").ap()
    tri_d = nc.dram_tensor("tri", [128, 128], BF16, kind="ExternalInput").ap()
    out_d = nc.dram_tensor("out", [T, D], BF16, kind="ExternalOutput").ap()

    with TileContext(nc) as tc:
        with (
            tc.tile_pool(name="const", bufs=1) as cpool,
            tc.tile_pool(name="big", bufs=1) as big,
            tc.tile_pool(name="rope", bufs=3) as rope,
            tc.tile_pool(name="expp", bufs=4) as expp,
            tc.tile_pool(name="denp", bufs=2) as denp,
            tc.tile_pool(name="outp", bufs=3) as outp,
            tc.tile_pool(name="ps_mm", bufs=2, space="PSUM") as ps_mm,
            tc.tile_pool(name="ps_sc", bufs=2, space="PSUM") as ps_sc,
            tc.tile_pool(name="ps_pv", bufs=1, space="PSUM") as ps_pv,
        ):
            cos_sb = cpool.tile([128, T], BF16)
            sin_sb = cpool.tile([128, T], BF16)
            tri_sb = cpool.tile([128, 128], BF16)
            xT_sb = big.tile([128, NT, KC, 512], BF16)
            wqk_sb = big.tile([128, KC, 2 * DG], BF16)
            wv_sb = big.tile([128, KC, DG], BF16)
            wout_sb = big.tile([128, 2, D], BF16)
            # first q/k matmuls need wqk + xT chunk 0; RoPE needs cos/sin;
            # v tiles need wv.  Later chunks + wout stream behind.
            nc.sync.dma_start(out=wqk_sb[:], in_=wqk_d[:])
            nc.sync.dma_start(out=xT_sb[:, 0], in_=xT_d[:, 0])
            nc.sync.dma_start(out=wv_sb[:], in_=wv_d[:])
            nc.sync.dma_start(out=cos_sb[:], in_=cos_d[:])
            nc.sync.dma_start(out=sin_sb[:], in_=sin_d[:])
            nc.sync.dma_start(out=tri_sb[:], in_=tri_d[:])
            for n in range(1, NT):
                nc.sync.dma_start(out=xT_sb[:, n], in_=xT_d[:, n])
            nc.sync.dma_start(out=wout_sb[:], in_=wout_d[:])

            # PE warm-up: dummy matmuls fill the DMA lead-in so HAM unthrottles
            # before the first real matmul
            warm_sb = cpool.tile([128, 256], BF16)
            nc.vector.memset(warm_sb[:], 0.0)
            for w in range(N_WARM):
                wp = ps_sc.tile([128, 256], F32, tag="sc")
                nc.tensor.matmul(
                    wp[:], lhsT=warm_sb[:, 0:128], rhs=warm_sb[:],
                    start=True, stop=True,
                )

            # qkT_sb m-index: 0,1 = q head-pairs (0,1),(2,3); 2,3 = k pairs
            qkT_sb = big.tile([128, 4, T], BF16)
            v_sb = big.tile([128, TT, HPG, HD + 1], BF16)
            nc.vector.memset(v_sb[:, :, :, HD], 1.0)
            outT_sb = big.tile([128, 2, T], BF16)

            def qk_tile(n, m):
                """q/k projection tile m (columns 512n:512n+512) + RoPE.
                Yields once mid-way (filler segmentation)."""
                ns = slice(n * 512, (n + 1) * 512)
                ps = ps_mm.tile([128, 512], F32, tag="mm")
                for k in range(KC):
                    nc.tensor.matmul(
                        ps[:],
                        lhsT=wqk_sb[:, k, m * 128:(m + 1) * 128],
                        rhs=xT_sb[:, n, k, :],
                        start=(k == 0),
                        stop=(k == KC - 1),
                    )
                    if k == 3:
                        yield
                # RoPE: rot = ps*cos + swap16(ps)*sin_signed
                qk_bf = rope.tile([128, 512], BF16, tag="qkbf")
                nc.scalar.copy(out=qk_bf[:], in_=ps[:])
                swp = rope.tile([128, 512], BF16, tag="swp")
                nc.vector.stream_shuffle(swp[:], qk_bf[:], SWAP16)
                m1 = rope.tile([128, 512], BF16, tag="m1")
                nc.vector.tensor_mul(m1[:], qk_bf[:], cos_sb[:, ns])
                m2 = rope.tile([128, 512], BF16, tag="m2")
                # keep GpSimd broadcast-only: mixing op types there forces a
                # ~6us ucode library swap at every den broadcast
                nc.vector.tensor_mul(m2[:], swp[:], sin_sb[:, ns])
                nc.vector.tensor_add(qkT_sb[:, m, ns], m1[:], m2[:])

            def v_tile(j):
                """v projection for T-tile j (natural layout)."""
                n = j // 4
                ps = ps_mm.tile([128, 256], F32, tag="mm")
                for k in range(KC):
                    nc.tensor.matmul(
                        ps[:],
                        lhsT=xT_sb[:, n, k, (j % 4) * 128:(j % 4 + 1) * 128],
                        rhs=wv_sb[:, k, :],
                        start=(k == 0),
                        stop=(k == KC - 1),
                    )
                vv = ps[:].rearrange("p (h d) -> p h d", h=HPG)
                nc.scalar.copy(out=v_sb[:, j, :, 0:HD], in_=vv)
                yield

            def proj_tile(t, nh):
                """out-projection partial for T-tile t, D-half nh."""
                ps = ps_mm.tile([128, 512], F32, tag="mm")
                for c in range(2):
                    nc.tensor.matmul(
                        ps[:],
                        lhsT=outT_sb[:, c, t * 128:(t + 1) * 128],
                        rhs=wout_sb[:, c, nh * 512:(nh + 1) * 512],
                        start=(c == 0),
                        stop=(c == 1),
                    )
                ot = outp.tile([128, 512], BF16, tag="ot")
                if t < 8 and (t + nh) % 2 == 0:
                    nc.scalar.copy(out=ot[:], in_=ps[:])
                else:
                    nc.vector.tensor_copy(ot[:], ps[:])
                nc.sync.dma_start(
                    out=out_d[t * 128:(t + 1) * 128, nh * 512:(nh + 1) * 512],
                    in_=ot[:],
                )
                yield

            fillers = deque()

            def drain(k):
                while k > 0 and fillers:
                    gen = fillers.popleft()
                    try:
                        next(gen)
                        fillers.appendleft(gen)
                    except StopIteration:
                        pass
                    k -= 1

            def drain_all():
                while fillers:
                    drain(1)

            def attn_group(g):
                """Attention for query block g (queries 512g:512g+512), with
                filler units interleaved between steps.  PV emission lags two
                steps behind scores so the PE never waits on exp, and filler
                draining is front-loaded so cross-group RoPE chains finish
                early and group boundaries have PE food during the den path."""
                steps = [(hp, j) for hp in range(2) for j in range(4 * g + 4)]
                quota, acc = len(fillers) / len(steps), 0.0
                pend = deque()
                pvt = {}
                for hp, j in steps:
                    if j == 0:
                        pvt[0] = ps_pv.tile([65, 512], F32, tag="pv0", name="pv0")
                        pvt[1] = ps_pv.tile([65, 512], F32, tag="pv1", name="pv1")
                    d = j - 4 * g
                    nstart = 128 * d if d > 0 else 0
                    ncols = 512 - nstart
                    jmax = 4 * g + 3
                    sc = ps_sc.tile([128, 1024], F32, tag="sc")
                    ex = expp.tile([128, 1024], BF16, tag="ex")
                    # two heads' score matmuls (row groups 0-1 / 2-3), one
                    # wide exp over both
                    for half in range(2):
                        pb = 64 * half
                        nc.tensor.matmul(
                            sc[:, half * 512:half * 512 + ncols],
                            lhsT=qkT_sb[pb:pb + 64, 2 + hp, j * 128:(j + 1) * 128],
                            rhs=qkT_sb[pb:pb + 64, hp, g * 512 + nstart:(g + 1) * 512],
                            start=True,
                            stop=True,
                        )
                    if ncols == 512:
                        nc.scalar.activation(
                            ex[:], sc[:],
                            mybir.ActivationFunctionType.Exp, scale=0.125,
                        )
                    else:
                        exv = ex[:].rearrange("p (u c) -> p u c", u=2)[:, :, 0:ncols]
                        scv = sc[:].rearrange("p (u c) -> p u c", u=2)[:, :, 0:ncols]
                        nc.scalar.activation(
                            exv, scv, mybir.ActivationFunctionType.Exp, scale=0.125,
                        )
                    if d >= 0:
                        nc.vector.tensor_mul(ex[:, 0:128], ex[:, 0:128], tri_sb[:])
                        nc.vector.tensor_mul(ex[:, 512:640], ex[:, 512:640], tri_sb[:])

                    acc += quota
                    while acc >= 1.0:
                        drain(1)
                        acc -= 1.0

                    def emit_pv(hp, j, ex, nstart, ncols):
                        for half in range(2):
                            nc.tensor.matmul(
                                pvt[half][:, nstart:512],
                                lhsT=v_sb[:, j, 2 * hp + half, :],
                                rhs=ex[:, half * 512:half * 512 + ncols],
                                start=(j == 0),
                                stop=(j == jmax),
                            )

                    if len(pend) == 2:
                        emit_pv(*pend.popleft())
                    pend.append((hp, j, ex, nstart, ncols))
                    if j == jmax:
                        while pend:
                            drain(1)
                            emit_pv(*pend.popleft())
                        # normalize: denominators sit in pv row 64 (ones col)
                        for half in range(2):
                            pb = 64 * half
                            pv = pvt[half]
                            den = denp.tile([1, 512], F32, tag="den")
                            nc.scalar.copy(out=den[:], in_=pv[64:65, :])
                            rec = denp.tile([1, 512], F32, tag="rec")
                            nc.vector.reciprocal_approx_fast(rec[:], den[:])
                            recb = denp.tile([64, 512], F32, tag="recb")
                            nc.gpsimd.partition_broadcast(recb[:], rec[0:1, :], channels=64)
                            nc.vector.tensor_mul(
                                outT_sb[pb:pb + 64, hp, g * 512:(g + 1) * 512],
                                pv[0:64, :],
                                recb[:],
                            )
                        drain(2)
                drain_all()

            # preamble: chunk-0 q/k pairs needed by attn0-hp0 first, then v 0-3
            # and the hp1 pairs (their PE work covers m0/m2's RoPE-chain latency)
            for gen in [qk_tile(0, 0), qk_tile(0, 2), v_tile(0), v_tile(1),
                        v_tile(2), v_tile(3), qk_tile(0, 1), qk_tile(0, 3)]:
                for _ in gen:
                    pass

            # attn0 fillers: chunk1 qk
            fillers.extend(qk_tile(1, m) for m in range(4))
            attn_group(0)

            # attn1 fillers: v 4-7 first (needed by attn1 pv), chunk2, proj0
            fillers.extend(v_tile(j) for j in range(4, 8))
            fillers.extend(qk_tile(2, m) for m in range(4))
            fillers.extend(proj_tile(t, nh) for t in range(0, 4) for nh in range(2))
            attn_group(1)

            # attn2 fillers: v 8-11, chunk3, proj1
            fillers.extend(v_tile(j) for j in range(8, 12))
            fillers.extend(qk_tile(3, m) for m in range(4))
            fillers.extend(proj_tile(t, nh) for t in range(4, 8) for nh in range(2))
            attn_group(2)

            # attn3 fillers: v 12-15, proj2, proj3
            fillers.extend(v_tile(j) for j in range(12, 16))
            fillers.extend(proj_tile(t, nh) for t in range(8, 12) for nh in range(2))
            attn_group(3)

            fillers.extend(proj_tile(t, nh) for t in range(12, 16) for nh in range(2))
            drain_all()

            if DEBUG_DUMPS:
                dq = nc.dram_tensor("dbg_qk", [128, 4, T], BF16, kind="ExternalOutput").ap()
                dv = nc.dram_tensor("dbg_v", [128, TT, HPG, HD + 1], BF16, kind="ExternalOutput").ap()
                do = nc.dram_tensor("dbg_outT", [128, 2, T], BF16, kind="ExternalOutput").ap()
                nc.sync.dma_start(out=dq[:], in_=qkT_sb[:])
                nc.sync.dma_start(out=dv[:], in_=v_sb[:])
                nc.sync.dma_start(out=do[:], in_=outT_sb[:])

    nc.compile()
    return nc


def _qk_perm():
    """hd permutation for q/k columns: RoPE pair j -> (e,o) rows 16-interleaved
    so the swap stays within 32-partition quadrants (stream_shuffle-able)."""
    perm = np.empty(HD, dtype=np.int64)
    for p in range(HD):
        q32, i = divmod(p, 32)
        j = 16 * q32 + (i % 16)
        perm[p] = 2 * j + (1 if i >= 16 else 0)
    return perm


def _prepare_shards(x, w_qkv, w_out, freqs_cos, freqs_sin):
    perm = _qk_perm()
    cosT = np.ascontiguousarray(freqs_cos.T)  # [32, T]
    sinT = np.ascontiguousarray(freqs_sin.T)
    # row p of a 64-row head block: pair j = 16*(p//32 % 2) + p%16, sign -/+ for e/o
    cos128 = np.empty((128, T), dtype=np.float32)
    sin128s = np.empty((128, T), dtype=np.float32)
    for p in range(128):
        ph = p % 64
        q32, i = divmod(ph, 32)
        j = 16 * q32 + (i % 16)
        cos128[p] = cosT[j]
        sin128s[p] = sinT[j] * (-1.0 if i < 16 else 1.0)
    kk, qq = np.meshgrid(np.arange(128), np.arange(128), indexing="ij")
    tri = (kk <= qq).astype(np.float32)

    w3 = w_qkv.reshape(D, 3, H, HD)
    in_maps = []
    for core in range(8):
        b, g = divmod(core, G)
        heads = np.arange(g * HPG, (g + 1) * HPG)
        wq = w3[:, 0, heads][:, :, perm].reshape(D, DG)
        wk = w3[:, 1, heads][:, :, perm].reshape(D, DG)
        wqk = np.ascontiguousarray(np.concatenate([wq, wk], axis=1))
        wv = np.ascontiguousarray(w3[:, 2, heads].reshape(D, DG))
        wo = np.ascontiguousarray(w_out.reshape(H, HD, D)[heads].reshape(DG, D))
        def sb_layout(a, kc=KC):
            # [128*kc, F] -> [128, kc, F] with partition-major contiguity
            return np.ascontiguousarray(a.reshape(kc, 128, -1).transpose(1, 0, 2))
        # xT chunks: [128, NT, KC, 512] so each chunk is one contiguous DMA
        xc = x[b].reshape(NT, 512, KC, 128).transpose(3, 0, 2, 1)
        in_maps.append({
            "xT": np.ascontiguousarray(xc).astype(BF16_NP),
            "wqk": sb_layout(wqk).astype(BF16_NP),
            "wv": sb_layout(wv).astype(BF16_NP),
            "wout": sb_layout(wo, kc=2).astype(BF16_NP),
            "cos128": cos128.astype(BF16_NP),
            "sin128s": sin128s.astype(BF16_NP),
            "tri": tri.astype(BF16_NP),
        })
    return in_maps


def _run(in_maps, **kw):
    if "nc" not in _CACHE:
        _CACHE["nc"] = _build()
    return run_bass_kernel_spmd(_CACHE["nc"], in_maps, core_ids=list(range(8)), **kw)


def kernel(x, w_qkv, w_out, freqs_cos, freqs_sin):
    x = np.asarray(x, dtype=np.float32)
    w_qkv = np.asarray(w_qkv, dtype=np.float32)
    w_out = np.asarray(w_out, dtype=np.float32)
    freqs_cos = np.asarray(freqs_cos, dtype=np.float32)
    freqs_sin = np.asarray(freqs_sin, dtype=np.float32)

    in_maps = _prepare_shards(x, w_qkv, w_out, freqs_cos, freqs_sin)
    res = _run(in_maps)
    out = np.zeros((B, T, D), dtype=np.float64)
    for core in range(8):
        out[core // G] += np.asarray(res.results[core]["out"]).astype(np.float64)
    return out.astype(np.float32)
